# revision 3
# baseline (speedup 1.0000x reference)
"""Context-aware attention pooling kernel for Trainium2 (8 NeuronCores).

Reference computation (per batch b):
    e      = tanh(seq @ W1[:256] + ctx @ W1[256:])      # [T, 64]
    logits = e @ W2                                      # [T, 1]
    a      = softmax(logits over T)
    out    = sum_t a[t] * seq[t]                         # [256]

Shapes: B=64, T=4096, D1=256, D2=128, UNITS=64.
Sharding: data-parallel over batch, 8 batches per core; W1/W2 replicated.

Per-core program (all t-tiles are 128 rows):
  - seq[b] loaded in natural layout [t, d] as bf16 (cast during SWDGE DMA),
    tile layout nat[p, n*256 + d] = seq[b, n*128+p, d]
  - PE transposes produce seqT_h[dh, t] = seq[b, t, 128h+dh] (bf16)
  - e-matmul contracts d on PE: eT[u, t] (tanh + ctx-bias fused on ScalarE)
  - logits via PE: lg[tau, n] = logit(t = 128n + tau) in PSUM [128, 32]
  - softmax without max-subtraction (|logit| <= ||W2||_1, safe in f32);
    Exp + row-sums fused on ScalarE; total Z via ones-matmul; the 1/Z
    scale is applied once to the pooled output
  - pooling on PE: p-columns stationary, natural seq tiles moving,
    accumulated over the 32 t-tiles into PSUM [1, 256]
"""

import numpy as np

import concourse.bass as bass
import concourse.bacc as bacc
import concourse.mybir as mybir
from concourse.tile import TileContext
from concourse import masks

F32 = mybir.dt.float32
BF16 = mybir.dt.bfloat16

N_CORES = 8
B_CORE = 8          # batches per core
T = 4096
D1 = 256
D2 = 128
U = 64
NT = T // 128       # 32 t-tiles per batch


def build_program():
    nc = bacc.Bacc("TRN2", target_bir_lowering=False, debug=False)

    seq = nc.declare_dram_parameter("seq", [B_CORE, T, D1], F32, isOutput=False)
    ctx = nc.declare_dram_parameter("ctx", [B_CORE, D2], F32, isOutput=False)
    w1 = nc.declare_dram_parameter("w1", [D1 + D2, U], F32, isOutput=False)
    w2 = nc.declare_dram_parameter("w2", [U, 1], F32, isOutput=False)
    outp = nc.declare_dram_parameter("outp", [1, B_CORE * D1], F32, isOutput=True)

    with TileContext(nc) as tc:
        with (
            tc.tile_pool(name="singles", bufs=1) as singles,
            tc.tile_pool(name="nat_pool", bufs=3) as nat_pool,
            tc.tile_pool(name="seqt_pool", bufs=2) as seqt_pool,
            tc.tile_pool(name="et_pool", bufs=2) as et_pool,
            tc.tile_pool(name="small_pool", bufs=2) as small_pool,
            tc.tile_pool(name="ps", bufs=1, space="PSUM") as ps,
        ):
            # ---- one-time setup ----
            ident = singles.tile([128, 128], BF16)
            masks.make_identity(nc, ident)
            ident8 = singles.tile([8, 8], F32)
            masks.make_identity(nc, ident8)

            w1s0 = singles.tile([128, U], BF16)
            nc.gpsimd.dma_start(out=w1s0, in_=w1[0:128, :])
            w1s1 = singles.tile([128, U], BF16)
            nc.gpsimd.dma_start(out=w1s1, in_=w1[128:256, :])
            w1c = singles.tile([128, U], F32)
            nc.sync.dma_start(out=w1c, in_=w1[256:384, :])
            w2t = singles.tile([U, 1], BF16)
            nc.gpsimd.dma_start(out=w2t, in_=w2[:, :])

            ctx_nat = singles.tile([B_CORE, D2], F32)
            nc.sync.dma_start(out=ctx_nat, in_=ctx[:, :])
            ctxT_ps = ps.tile([D2, B_CORE], F32, tag="cb", bufs=1)
            nc.tensor.transpose(ctxT_ps, ctx_nat, ident8)
            ctxT = singles.tile([D2, B_CORE], F32)
            nc.vector.tensor_copy(ctxT, ctxT_ps)

            ones_col = singles.tile([128, 1], F32)
            nc.vector.memset(ones_col, 1.0)

            final_sb = singles.tile([1, B_CORE * D1], F32)

            # ---- per-batch pipeline ----
            for b in range(B_CORE):
                # natural-layout bf16 load (cast during DMA)
                nat = nat_pool.tile([128, NT * D1], BF16, tag="nat")
                nc.gpsimd.dma_start(
                    out=nat.rearrange("p (n d) -> p n d", d=D1),
                    in_=seq[b].rearrange("(n p) d -> p n d", p=128),
                )

                # context projection -> bias column [64, 1] (f32 path)
                cb_ps = ps.tile([U, 1], F32, tag="cb", bufs=1)
                nc.tensor.matmul(
                    cb_ps, lhsT=w1c, rhs=ctxT[:, b : b + 1], start=True, stop=True
                )
                cb_sb = small_pool.tile([U, 1], F32, tag="cb_sb")
                nc.scalar.copy(cb_sb, cb_ps)

                # PE transposes: nat [t, d] blocks -> seqT_h [dh, t]
                seqT0 = seqt_pool.tile([128, T], BF16, tag="seqT0")
                seqT1 = seqt_pool.tile([128, T], BF16, tag="seqT1")
                for h, seqT in ((0, seqT0), (1, seqT1)):
                    for k in range(NT // 4):
                        pst = ps.tile([128, 512], BF16, tag="tp", bufs=2)
                        for i in range(4):
                            n = 4 * k + i
                            nc.tensor.transpose(
                                pst[:, 128 * i : 128 * (i + 1)],
                                nat[:, 256 * n + 128 * h : 256 * n + 128 * h + 128],
                                ident,
                            )
                        nc.vector.tensor_copy(seqT[:, 512 * k : 512 * (k + 1)], pst)

                # e = tanh(z + cb), computed as eT [64, t] in bf16
                eT = et_pool.tile([U, T], BF16, tag="eT")
                for c in range(T // 512):
                    e_ps = ps.tile([U, 512], F32, tag="e", bufs=2)
                    sl = slice(512 * c, 512 * (c + 1))
                    nc.tensor.matmul(
                        e_ps, lhsT=w1s0, rhs=seqT0[:, sl], start=True, stop=False
                    )
                    nc.tensor.matmul(
                        e_ps, lhsT=w1s1, rhs=seqT1[:, sl], start=False, stop=True
                    )
                    nc.scalar.activation(
                        eT[:, sl],
                        e_ps,
                        mybir.ActivationFunctionType.Tanh,
                        bias=cb_sb,
                    )

                # logits columns: lg[tau, n] = logit(t = 128n + tau)
                lg_ps = ps.tile([128, NT], F32, tag="lg", bufs=1)
                for n in range(NT):
                    nc.tensor.matmul(
                        lg_ps[:, n : n + 1],
                        lhsT=eT[:, 128 * n : 128 * (n + 1)],
                        rhs=w2t,
                        start=True,
                        stop=True,
                    )

                # p = exp(logits) with fused per-partition sums
                p_sb = small_pool.tile([128, NT], BF16, tag="p")
                psums = small_pool.tile([128, 1], F32, tag="psums")
                nc.scalar.activation(
                    p_sb, lg_ps, mybir.ActivationFunctionType.Exp, accum_out=psums
                )

                # Z = sum over partitions of psums
                z_ps = ps.tile([1, 1], F32, tag="z", bufs=1)
                nc.tensor.matmul(z_ps, lhsT=psums, rhs=ones_col, start=True, stop=True)
                invz = small_pool.tile([1, 1], F32, tag="invz")
                nc.vector.reciprocal(invz, z_ps)

                # pooling: out[d] = sum_t p[t] * seq[t, d], accumulated on PE
                pool_ps = ps.tile([1, D1], F32, tag="pool", bufs=1)
                for n in range(NT):
                    nc.tensor.matmul(
                        pool_ps,
                        lhsT=p_sb[:, n : n + 1],
                        rhs=nat[:, 256 * n : 256 * (n + 1)],
                        start=(n == 0),
                        stop=(n == NT - 1),
                    )

                # normalize by 1/Z while evacuating to SBUF
                nc.scalar.activation(
                    final_sb[0:1, D1 * b : D1 * (b + 1)],
                    pool_ps,
                    mybir.ActivationFunctionType.Copy,
                    scale=invz,
                )

            nc.sync.dma_start(out=outp[:, :], in_=final_sb)

    nc.compile()
    return nc


_NC_CACHE = []


def _get_program():
    if not _NC_CACHE:
        _NC_CACHE.append(build_program())
    return _NC_CACHE[0]


def kernel(sequence, context, W1, W2):
    """Full-input entry point: shards batch across 8 cores, returns [64, 256] f32."""
    from concourse.bass_utils import run_bass_kernel_spmd

    nc = _get_program()
    in_maps = []
    for c in range(N_CORES):
        sl = slice(B_CORE * c, B_CORE * (c + 1))
        in_maps.append(
            {
                "seq": np.ascontiguousarray(sequence[sl], dtype=np.float32),
                "ctx": np.ascontiguousarray(context[sl], dtype=np.float32),
                "w1": np.ascontiguousarray(W1, dtype=np.float32),
                "w2": np.ascontiguousarray(W2, dtype=np.float32),
            }
        )
    res = run_bass_kernel_spmd(nc, in_maps, list(range(N_CORES)))
    out = np.concatenate(
        [res.results[c]["outp"].reshape(B_CORE, D1) for c in range(N_CORES)], axis=0
    )
    return out.astype(np.float32)
